# revision 15
# baseline (speedup 1.0000x reference)
"""Trainium2 Bass kernel for nn_CombinedConsecutiveAdjustment (B=8192, S=4096).

Math reduction of the reference
-------------------------------
With g in {0,1}:
  - eye_streaks = cumsum(g)*g, so max(eye_streaks) is the total count of ones
    N1, and argmax is the index of the LAST one, pos (0 if the row is all
    zeros, matching jnp.argmax of an all-zero vector).
  - att_streaks' max is the run of zeros after pos: T = S-1-pos when N1>0.
    (Reference yields -inf when pos==S-1; T=0 fails the >=40 gate identically,
    and the N1==0 case is gated out by N1>=40.)
  - adjustment = (N1>=EYE_TH and T>=ATT_TH) ? MAX_ADJ*(1-exp(-(T-ATT_TH)*3/SAT)) : 0
    where the T>=ATT_TH gate folds into max(adjraw, 0): for T<ATT_TH the raw
    exp formula goes negative and at T==ATT_TH it is exactly 0.
  - out = clip(d*(1-adjustment), MIN_OUT, MAX_OUT)
So per row only two reductions are needed: N1 = sum(g) and pos = max_j(j*g[j]).

Distribution (pure data parallel, per the sharding hint)
--------------------------------------------------------
1024 rows per core on 8 cores. Per core, row r -> (partition p=r//8,
column t=r%8); every DMA descriptor is a contiguous 8KB DRAM read.

Schedule (HBM stream is the roofline: 16.8MB/core ~= 46.7us at 360GB/s):
  - 2048-column chunks: DMA cadence 2.9us/chunk while ACT needs 1.9us
    (i32->i16 Copy cast) and DVE needs 2.3us (tensor_tensor mult-by-iota at
    2x + two tensor_scalar accumulations at 4x), so both compute engines
    catch up on the 900ns DMA-completion semaphore lag instead of
    accumulating a backlog toward the stream end.
  - Tile 7 streams as shrinking chunks (2048..32); the final 32-column
    chunk bypasses ACT entirely (DVE does cast+count+pos straight from the
    i32 data), minimizing the post-stream dependency chain.
  - Per-chunk partials land in memset-0 accumulator columns; one
    tensor_reduce per quantity folds them at the end.
  - Epilogue on [128, 8]: Exp taken directly as exp((pos-C)*3/SAT) via
    activation scale+bias, gated by (N1>=EYE_TH) and max(.,0), then
    out = clip(d + d*(-adj)).
"""

import numpy as np

B = 8192
S = 4096
N_CORES = 8
BC = B // N_CORES          # rows per core = 1024
TILES = BC // 128          # partition tiles per core = 8
CHUNK = 2048

EYE_TH = 40.0
ATT_TH = 40.0
MAX_ADJ = 0.05
SAT = 160.0
MIN_OUT = 0.01
MAX_OUT = 1.0

# tile-7 chunk split: the big head chunks stream FIRST (chunk order is
# free -- only the accumulators care about (tile, chunk) identity), the
# small ones go at the very end so the post-stream drain is short. The
# last chunk is tiny and DVE-only. ACT has ~430ns fixed cost per
# instruction, so the tail must not over-fragment.
T7_HEAD = ()
T7_TAIL = (1024, 1024, 1024, 512, 256, 128, 96, 32)
assert sum(T7_HEAD) + sum(T7_TAIL) == S
T7_CHUNKS = T7_HEAD + T7_TAIL
ACC_K = max(S // CHUNK, len(T7_CHUNKS))   # accumulator columns per tile

_CACHE = {}


def _build(gbufs=6, fbufs=4, pbufs=3, iota_splits=2,
           t7_head=T7_HEAD, t7_tail=T7_TAIL):
    import concourse.bacc as bacc
    import concourse.tile as tile
    import concourse.mybir as mybir

    t7_chunks = tuple(t7_head) + tuple(t7_tail)
    assert sum(t7_chunks) == S
    acc_k = max(S // CHUNK, len(t7_chunks))

    nc = bacc.Bacc(
        "TRN2",
        target_bir_lowering=False,
        debug=False,
        num_devices=N_CORES,
    )
    f32 = mybir.dt.float32
    i32 = mybir.dt.int32
    i16 = mybir.dt.int16

    g_dram = nc.dram_tensor("g", [BC, S], i32, kind="ExternalInput").ap()
    d_dram = nc.dram_tensor("d", [BC, 1], f32, kind="ExternalInput").ap()
    o_dram = nc.dram_tensor("o", [BC, 1], f32, kind="ExternalOutput").ap()

    g_view = g_dram.rearrange("(p t) s -> t p s", t=TILES)    # [t][128, s]
    d_view = d_dram.rearrange("(p t) o -> p (t o)", t=TILES)  # [128, tiles]
    o_view = o_dram.rearrange("(p t) o -> p (t o)", t=TILES)  # [128, tiles]

    Copy = mybir.ActivationFunctionType.Copy
    Exp = mybir.ActivationFunctionType.Exp
    A = mybir.AluOpType

    # (tile, lo, n, k) schedule: tile-7 head chunks first, then tiles 0..6
    # as uniform 2048 chunks, then tile-7's shrinking tail
    t7 = []
    lo = 0
    for c, n in enumerate(t7_chunks):
        t7.append((TILES - 1, lo, n, c))
        lo += n
    nhead = len(t7_head)
    sched = list(t7[:nhead])
    for t in range(TILES - 1):
        for k in range(S // CHUNK):
            sched.append((t, k * CHUNK, CHUNK, k))
    sched += t7[nhead:]

    with tile.TileContext(nc) as tc:
        with (
            tc.tile_pool(name="gpool", bufs=gbufs) as gpool,
            tc.tile_pool(name="fpool", bufs=fbufs) as fpool,
            tc.tile_pool(name="ppool", bufs=pbufs) as ppool,
            tc.tile_pool(name="small", bufs=1) as small,
        ):
            # iota carrying global column values, emitted in pieces so the
            # first chunk's compute never waits on the whole table
            iota = small.tile([128, S], i16)
            istep = S // iota_splits
            for k in range(iota_splits):
                nc.gpsimd.iota(iota[:, k * istep : (k + 1) * istep],
                               pattern=[[1, istep]], base=k * istep,
                               channel_multiplier=0)

            # per-partition bias for the fused Exp (registered const pool
            # only carries 0.0/1.0)
            CPOS = float(S - 1 - ATT_TH)
            ebias = small.tile([128, 1], f32)
            nc.gpsimd.memset(ebias[:], -3.0 * CPOS / SAT)

            pos_acc = small.tile([128, TILES * acc_k], f32)
            cnt_acc = small.tile([128, TILES * acc_k], f32)
            nc.gpsimd.memset(pos_acc[:], 0.0)
            nc.gpsimd.memset(cnt_acc[:], 0.0)
            d_sb = small.tile([128, TILES], f32)

            first = True
            for t, lo, n, k in sched:
                col = t * acc_k + k
                t7 = t == TILES - 1
                last = t7 and k == len(t7_chunks) - 1
                # tile-7 chunks get dedicated buffers: their DMAs must never
                # wait on earlier chunks' compute for a slot at the tail
                if t7:
                    gt = small.tile([128, n], i32, name=f"t7g{k}")
                else:
                    gt = gpool.tile([128, CHUNK], i32, name="gt")[:, :n]
                nc.sync.dma_start(out=gt[:], in_=g_view[t][:, lo : lo + n])
                if first:
                    # tiny, issued right after the first big DMA
                    nc.sync.dma_start(out=d_sb[:], in_=d_view)
                    first = False
                prod = ppool.tile([128, CHUNK], i16, name="prod")
                if t7:
                    gf = small.tile([128, n], i16, name=f"t7f{k}")
                else:
                    gf = fpool.tile([128, CHUNK], i16, name="gf")[:, :n]
                nc.scalar.activation(out=gf[:], in_=gt[:], func=Copy,
                                     accum_out=cnt_acc[:, col : col + 1])
                nc.vector.tensor_tensor(out=prod[:, :n], in0=gf[:],
                                        in1=iota[:, lo : lo + n], op=A.mult)
                nc.vector.tensor_scalar(out=prod[:, :n], in0=prod[:, :n],
                                        scalar1=0, scalar2=None,
                                        op0=A.max, op1=A.max,
                                        accum_out=pos_acc[:, col : col + 1])

            # ---- fold partials, then the [128, tiles] epilogue ----
            pos_f = small.tile([128, TILES], f32)
            cnt_f = small.tile([128, TILES], f32)
            nc.vector.tensor_reduce(
                pos_f[:], pos_acc[:].rearrange("p (t k) -> p t k", k=acc_k),
                axis=mybir.AxisListType.X, op=A.max)

            e_f = small.tile([128, TILES], f32)
            conda = small.tile([128, TILES], f32)
            adjraw = small.tile([128, TILES], f32)
            c1 = small.tile([128, TILES], f32)
            negadj = small.tile([128, TILES], f32)
            da = small.tile([128, TILES], f32)
            outp = small.tile([128, TILES], f32)
            res = small.tile([128, TILES], f32)

            # e = exp((pos - (S-1-ATT_TH)) * 3/SAT)  (ACT; overlaps DVE below)
            nc.scalar.activation(out=e_f[:], in_=pos_f[:], func=Exp,
                                 scale=3.0 / SAT, bias=ebias[:])
            # count fold + gate run on DVE while ACT does the Exp
            nc.vector.tensor_reduce(
                cnt_f[:], cnt_acc[:].rearrange("p (t k) -> p t k", k=acc_k),
                axis=mybir.AxisListType.X, op=A.add)
            nc.vector.tensor_scalar(out=conda[:], in0=cnt_f[:],
                                    scalar1=EYE_TH, scalar2=None, op0=A.is_ge)
            # adjraw = MAX_ADJ - MAX_ADJ*e   (negative when T < ATT_TH)
            nc.vector.tensor_scalar(out=adjraw[:], in0=e_f[:],
                                    scalar1=-MAX_ADJ, scalar2=MAX_ADJ,
                                    op0=A.mult, op1=A.add)
            nc.vector.tensor_tensor(out=c1[:], in0=adjraw[:], in1=conda[:],
                                    op=A.mult)
            # negadj = -max(c1, 0)
            nc.vector.tensor_scalar(out=negadj[:], in0=c1[:],
                                    scalar1=0.0, scalar2=-1.0,
                                    op0=A.max, op1=A.mult)
            nc.vector.tensor_tensor(out=da[:], in0=negadj[:], in1=d_sb[:],
                                    op=A.mult)
            nc.vector.tensor_tensor(out=outp[:], in0=d_sb[:], in1=da[:],
                                    op=A.add)
            nc.vector.tensor_scalar(out=res[:], in0=outp[:],
                                    scalar1=MIN_OUT, scalar2=MAX_OUT,
                                    op0=A.max, op1=A.min)
            nc.sync.dma_start(out=o_view, in_=res[:])

    nc.compile()
    return nc


def _get_nc(**kw):
    key = tuple(sorted(kw.items()))
    if key not in _CACHE:
        _CACHE[key] = _build(**kw)
    return _CACHE[key]


def kernel(drowsiness_index, gesture_sequence):
    from concourse.bass_utils import run_bass_kernel_spmd

    d = np.asarray(drowsiness_index, dtype=np.float32).reshape(B, 1)
    g = np.ascontiguousarray(np.asarray(gesture_sequence, dtype=np.int32).reshape(B, S))

    nc = _get_nc()
    in_maps = [
        {"g": g[c * BC : (c + 1) * BC], "d": d[c * BC : (c + 1) * BC]}
        for c in range(N_CORES)
    ]
    r = run_bass_kernel_spmd(nc, in_maps, list(range(N_CORES)))
    out = np.concatenate([r.results[c]["o"] for c in range(N_CORES)], axis=0)
    return out.reshape(B, 1).astype(np.float32, copy=False)
